# revision 15
# baseline (speedup 1.0000x reference)
"""Cosine-similarity attention map on 8 Trainium2 NeuronCores.

out[b, i, j] = <x[b,:,i], x[b,:,j]> / (||x[b,:,i]|| * ||x[b,:,j]||)
x: [B=4, C=64, N=4096] fp32  ->  out: [B=4, N=4096, N=4096] fp32

The output is a symmetric Gram matrix, so each core computes only its
share of the (block) upper triangle, in fp16, and the host mirrors the
lower triangle while unsharding (rel tolerance 2e-2; fp16 costs ~5e-4).

Sharding: 2 cores per batch. Global 128-row panels t = 0..31 of out[b];
core r in {0,1} owns panels t = 2p + r (p = 0..15 local). Panel t only
needs columns >= 128t; each slab starts exactly at its diagonal (shifted
col 256p) — identical per-panel shapes for both cores, so one SPMD
program serves all 8. Row data comes from the
same normalized tensor as column data (rows == cols of a Gram matrix):
core r receives x[b] rolled left by 128*r columns. The channel rows are
host-duplicated to K=128 ([x; x] doubles sumsq, and rsqrt then yields
exactly the extra 1/sqrt(2) each copy needs): a full-height contraction
keeps the PE's HAM activity monitor in its warm state (2.4 GHz); a
12-matmul dependency-free burst warms it up front.
"""

import sys

sys.path.insert(0, "/opt/trn_rl_repo")

import numpy as np

import concourse.bass as bass
import concourse.mybir as mybir
import concourse.tile as tile
from concourse import bacc
from concourse.bass_utils import run_bass_kernel_spmd

B, C, N = 4, 64, 4096
NCORES = 8
RB = 2048  # 16 local 128-row panels per core
CH = 512  # norm / matmul column chunk
NCH = N // CH  # 8
GW = 2  # chunks per PSUM group (copy width 1024)

F32 = mybir.dt.float32
F16 = mybir.dt.float16


def _build():
    nc = bacc.Bacc("TRN2", target_bir_lowering=False)
    xf = nc.declare_dram_parameter("xf", [2 * C, N], F16, isOutput=False)
    out = nc.declare_dram_parameter("out", [RB, N], F16, isOutput=True)

    # Projected busy time (us) per copy engine; DVE (0.96 GHz) also does
    # the 8 chunk muls, ACT (1.2 GHz) the 8 abs_rsqrt.
    eng_t = {"v": 5.5, "a": 6.0}

    with tile.TileContext(nc) as tc:
        with (
            tc.tile_pool(name="persist", bufs=1) as persist,
            tc.tile_pool(name="panels", bufs=6) as panels,
            tc.tile_pool(name="mpsum", bufs=3, space="PSUM") as mpsum,
            tc.tile_pool(name="npsum", bufs=2, space="PSUM") as npsum,
        ):
            # PE warm-up: 12 dependency-free matmuls (~7us cold, two full HAM
            # windows) flip the clock gate to 2.4 GHz before the real matmuls.
            # They use the first mpsum ring slot before any panel claims it.
            GARB = persist.tile([2 * C, CH], F16)
            nc.vector.memset(GARB, 0.5)
            WPS = mpsum.tile([128, GW * CH], F32, tag="ps")
            for _ in range(12):
                nc.tensor.matmul(
                    WPS[:, 0:CH], lhsT=GARB[:, 0:128], rhs=GARB, start=True, stop=True
                )

            XF = persist.tile([2 * C, N], F16)
            # Chunks are consumed descending (small panels first), so load
            # them in that order too, two chunks per DMA.
            for c in range(NCH - 2, -1, -2):
                cs = slice(c * CH, (c + 2) * CH)
                nc.sync.dma_start(out=XF[:, cs], in_=xf[:, cs])

            ones_f = persist.tile([2 * C, 1], F32)
            nc.vector.memset(ones_f, 1.0)
            ones_c = persist.tile([2 * C, 1], F16)  # sumsq reduction lhsT
            nc.vector.tensor_copy(ones_c, ones_f)
            ones_rf = persist.tile([1, 2 * C], F32)
            nc.vector.memset(ones_rf, 1.0)
            ones_r = persist.tile([1, 2 * C], F16)  # K=1 partition-broadcast lhsT
            nc.vector.tensor_copy(ones_r, ones_rf)

            SQ = persist.tile([2 * C, N], F16)
            RN16 = persist.tile([1, N], F16)
            YF = persist.tile([2 * C, N], F16)

            # x^2 on the otherwise-idle GpSimd, descending, as loads land.
            for c in range(NCH - 1, -1, -1):
                cs = slice(c * CH, (c + 1) * CH)
                nc.gpsimd.tensor_mul(SQ[:, cs], XF[:, cs], XF[:, cs])

            # Normalize columns of one 512-col chunk: y = x * rsqrt(sumsq).
            def norm_chunk(c):
                cs = slice(c * CH, (c + 1) * CH)
                pps = npsum.tile([128, CH], F32, tag="pps")
                nc.tensor.matmul(
                    pps[0:1, :], lhsT=ones_c, rhs=SQ[:, cs], start=True, stop=True
                )
                nc.scalar.activation(
                    RN16[:, cs],
                    pps[0:1, :],
                    mybir.ActivationFunctionType.Abs_reciprocal_sqrt,
                )
                nc.tensor.matmul(
                    pps[0 : 2 * C, :], lhsT=ones_r, rhs=RN16[:, cs], start=True, stop=True
                )
                nc.vector.tensor_mul(YF[:, cs], XF[:, cs], pps[0 : 2 * C, :])

            # PSUM -> SBUF evacuation, fp16 out (int8 measured SLOWER here:
            # the 8-bit store path runs below the 1 elem/cycle cast rate).
            def quant_copy(dst, src, cols):
                if eng_t["v"] <= eng_t["a"]:
                    eng_t["v"] += 0.105 + cols / 1010.0
                    nc.vector.tensor_copy(dst, src)
                else:
                    eng_t["a"] += 0.125 + cols / 1030.0
                    nc.scalar.copy(out=dst, in_=src)

            # Panels 2c and 2c+1: each slab starts exactly at its diagonal
            # (shifted col 256p; the 128-col roll makes this exact for odd
            # cores too). Odd panels lead with a 256-col segment placed at
            # PSUM offset 256 so every matmul output stays inside one bank.
            def emit_panels(c):
                for p in (2 * c, 2 * c + 1):
                    start = 256 * p
                    segs = []
                    a = start
                    if a % CH:
                        segs.append((a, CH - a % CH))
                        a += CH - a % CH
                    while a < N:
                        segs.append((a, CH))
                        a += CH
                    groups = []  # (psum_offset, [(abs_col, width), ...])
                    i = 0
                    if segs and segs[0][1] == 256:
                        n0 = min(2, len(segs))
                        groups.append((256, segs[0:n0]))
                        i = n0
                    while i < len(segs):
                        g = segs[i : i + GW]
                        groups.append((0, g))
                        i += len(g)
                    pnl = panels.tile([128, N], F16, tag="panel")
                    rs_ = slice(128 * p, 128 * (p + 1))
                    lhsT = YF[:, start : start + 128]
                    flush0 = 0
                    pend = 0
                    for gi, (psoff, g) in enumerate(groups):
                        ps = mpsum.tile([128, GW * CH], F32, tag="ps")
                        off = psoff
                        for aw in g:
                            nc.tensor.matmul(
                                ps[:, off : off + aw[1]],
                                lhsT=lhsT,
                                rhs=YF[:, aw[0] : aw[0] + aw[1]],
                                start=True,
                                stop=True,
                            )
                            off += aw[1]
                        gw = off - psoff
                        gl0 = g[0][0] - start
                        quant_copy(pnl[:, gl0 : gl0 + gw], ps[:, psoff:off], gw)
                        pend += gw
                        if pend >= 3 * CH or gi == len(groups) - 1:
                            fl = slice(flush0, flush0 + pend)
                            nc.sync.dma_start(out=out[rs_, fl], in_=pnl[:, fl])
                            flush0 += pend
                            pend = 0

            # Software-pipelined: panels for chunk c are emitted after the
            # norm of chunk c-1 so the norm chain never queues behind the
            # bulk matmul/copy work on DVE/ACT/PE.
            norm_chunk(NCH - 1)
            for c in range(NCH - 2, -1, -1):
                norm_chunk(c)
                emit_panels(c + 1)
            emit_panels(0)

    nc.compile()
    return nc


def _install_profile_hook():
    """This container's antenv lacks axon_hooks, so run_bass_kernel_spmd's
    trace=True path dies on import. Recreate the module and register the
    ctypes NTFF hook that trn_boot would have installed."""
    import sys as _sys
    import types

    if "antenv.axon_hooks" in _sys.modules:
        return
    import antenv

    mod = types.ModuleType("antenv.axon_hooks")
    mod._hook = None

    def set_axon_ntff_profile_hook(h):
        mod._hook = h

    def get_axon_ntff_profile_hook():
        return mod._hook

    mod.set_axon_ntff_profile_hook = set_axon_ntff_profile_hook
    mod.get_axon_ntff_profile_hook = get_axon_ntff_profile_hook
    _sys.modules["antenv.axon_hooks"] = mod
    antenv.axon_hooks = mod

    from trn_agent_boot.trn_boot import _ntff_profile_via_ctypes

    mod.set_axon_ntff_profile_hook(
        _ntff_profile_via_ctypes("/opt/axon/libaxon_pjrt.so")
    )


_nc = None


def _get_nc():
    global _nc
    if _nc is None:
        _nc = _build()
    return _nc


def _run(x, trace=False, trace_cores=None):
    x = np.asarray(x, dtype=np.float32)
    assert x.shape == (B, C, N), x.shape
    core_ids = list(range(NCORES))
    in_maps = []
    for k in core_ids:
        b, r = divmod(k, 2)
        xb = x[b] if r == 0 else np.roll(x[b], -128, axis=1)
        xb16 = xb.astype(np.float16)
        in_maps.append({"xf": np.ascontiguousarray(np.vstack([xb16, xb16]))})
    if trace:
        _install_profile_hook()
    res = run_bass_kernel_spmd(
        _get_nc(), in_maps, core_ids, trace=trace, trace_cores=trace_cores
    )
    out = np.empty((B, N, N), dtype=np.float32)
    for k in core_ids:
        b, r = divmod(k, 2)
        S = res.results[k]["out"]  # [2048, 4096] fp16
        Sf = S.astype(np.float32)
        for p in range(16):
            t = 2 * p + r
            L = N - 128 * t  # slab starts exactly at the diagonal
            out[b, 128 * t : 128 * (t + 1), 128 * t : 128 * t + L] = Sf[
                128 * p : 128 * (p + 1), 0:L
            ]
    # Mirror the block lower triangle from the computed upper part.
    for b in range(B):
        ob = out[b]
        for t in range(1, 32):
            fs = 128 * t
            if fs:
                ob[128 * t : 128 * (t + 1), 0:fs] = ob[
                    0:fs, 128 * t : 128 * (t + 1)
                ].T
    return out, res


def kernel(x):
    return _run(x)[0]


# revision 16
# speedup vs baseline: 1.0679x; 1.0679x over previous
"""Cosine-similarity attention map on 8 Trainium2 NeuronCores.

out[b, i, j] = <x[b,:,i], x[b,:,j]> / (||x[b,:,i]|| * ||x[b,:,j]||)
x: [B=4, C=64, N=4096] fp32  ->  out: [B=4, N=4096, N=4096] fp32

The output is a symmetric Gram matrix, so each core computes only its
share of the (block) upper triangle, in fp16, and the host mirrors the
lower triangle while unsharding (rel tolerance 2e-2; fp16 costs ~5e-4).

Sharding: 2 cores per batch. Global 128-row panels t = 0..31 of out[b];
core r in {0,1} owns panels t = 2p + r (p = 0..15 local). Panel t only
needs columns >= 128t; rounding down to 512-col chunks, local panel p
computes chunks floor(p/2)..7, width w = 8 - floor(p/2) — identical for
both cores, so one SPMD program serves all 8. Row data comes from the
same normalized tensor as column data (rows == cols of a Gram matrix):
core r receives x[b] rolled left by 128*r columns. The channel rows are
host-duplicated to K=128 ([x; x] doubles sumsq, and rsqrt then yields
exactly the extra 1/sqrt(2) each copy needs): a full-height contraction
keeps the PE's HAM activity monitor in its warm state (2.4 GHz); a
12-matmul dependency-free burst warms it up front.
"""

import sys

sys.path.insert(0, "/opt/trn_rl_repo")

import numpy as np

import concourse.bass as bass
import concourse.mybir as mybir
import concourse.tile as tile
from concourse import bacc
from concourse.bass_utils import run_bass_kernel_spmd

B, C, N = 4, 64, 4096
NCORES = 8
RB = 2048  # 16 local 128-row panels per core
CH = 512  # norm / matmul column chunk
NCH = N // CH  # 8
GW = 2  # chunks per PSUM group (copy width 1024)

F32 = mybir.dt.float32
F16 = mybir.dt.float16


def _build():
    nc = bacc.Bacc("TRN2", target_bir_lowering=False)
    xf = nc.declare_dram_parameter("xf", [2 * C, N], F16, isOutput=False)
    out = nc.declare_dram_parameter("out", [RB, N], F16, isOutput=True)

    # Projected busy time (us) per copy engine; DVE (0.96 GHz) also does
    # the 8 chunk muls, ACT (1.2 GHz) the 8 abs_rsqrt.
    eng_t = {"v": 5.5, "a": 6.0}

    with tile.TileContext(nc) as tc:
        with (
            tc.tile_pool(name="persist", bufs=1) as persist,
            tc.tile_pool(name="panels", bufs=6) as panels,
            tc.tile_pool(name="mpsum", bufs=3, space="PSUM") as mpsum,
            tc.tile_pool(name="npsum", bufs=2, space="PSUM") as npsum,
        ):
            # PE warm-up: 12 dependency-free matmuls (~7us cold, two full HAM
            # windows) flip the clock gate to 2.4 GHz before the real matmuls.
            # They use the first mpsum ring slot before any panel claims it.
            GARB = persist.tile([2 * C, CH], F16)
            nc.vector.memset(GARB, 0.5)
            WPS = mpsum.tile([128, GW * CH], F32, tag="ps")
            for _ in range(12):
                nc.tensor.matmul(
                    WPS[:, 0:CH], lhsT=GARB[:, 0:128], rhs=GARB, start=True, stop=True
                )

            XF = persist.tile([2 * C, N], F16)
            # Chunks are consumed descending (small panels first), so load
            # them in that order too, two chunks per DMA.
            for c in range(NCH - 2, -1, -2):
                cs = slice(c * CH, (c + 2) * CH)
                nc.sync.dma_start(out=XF[:, cs], in_=xf[:, cs])

            ones_f = persist.tile([2 * C, 1], F32)
            nc.vector.memset(ones_f, 1.0)
            ones_c = persist.tile([2 * C, 1], F16)  # sumsq reduction lhsT
            nc.vector.tensor_copy(ones_c, ones_f)
            ones_rf = persist.tile([1, 2 * C], F32)
            nc.vector.memset(ones_rf, 1.0)
            ones_r = persist.tile([1, 2 * C], F16)  # K=1 partition-broadcast lhsT
            nc.vector.tensor_copy(ones_r, ones_rf)

            SQ = persist.tile([2 * C, N], F16)
            RN16 = persist.tile([1, N], F16)
            YF = persist.tile([2 * C, N], F16)

            # x^2 on the otherwise-idle GpSimd, descending, as loads land.
            for c in range(NCH - 1, -1, -1):
                cs = slice(c * CH, (c + 1) * CH)
                nc.gpsimd.tensor_mul(SQ[:, cs], XF[:, cs], XF[:, cs])

            # Normalize columns of one 512-col chunk: y = x * rsqrt(sumsq).
            def norm_chunk(c):
                cs = slice(c * CH, (c + 1) * CH)
                pps = npsum.tile([128, CH], F32, tag="pps")
                nc.tensor.matmul(
                    pps[0:1, :], lhsT=ones_c, rhs=SQ[:, cs], start=True, stop=True
                )
                nc.scalar.activation(
                    RN16[:, cs],
                    pps[0:1, :],
                    mybir.ActivationFunctionType.Abs_reciprocal_sqrt,
                )
                nc.tensor.matmul(
                    pps[0 : 2 * C, :], lhsT=ones_r, rhs=RN16[:, cs], start=True, stop=True
                )
                nc.vector.tensor_mul(YF[:, cs], XF[:, cs], pps[0 : 2 * C, :])

            # PSUM -> SBUF evacuation, fp16 out (int8 measured SLOWER here:
            # the 8-bit store path runs below the 1 elem/cycle cast rate).
            def quant_copy(dst, src, cols):
                if eng_t["v"] <= eng_t["a"]:
                    eng_t["v"] += 0.105 + cols / 1010.0
                    nc.vector.tensor_copy(dst, src)
                else:
                    eng_t["a"] += 0.125 + cols / 1030.0
                    nc.scalar.copy(out=dst, in_=src)

            # Panels 2c and 2c+1: rhs chunks c..7, lhsT inside chunk c.
            def emit_panels(c):
                js = list(range(c, NCH))
                groups = [js[i : i + GW] for i in range(0, len(js), GW)]
                for p in (2 * c, 2 * c + 1):
                    pnl = panels.tile([128, N], F16, tag="panel")
                    rs_ = slice(128 * p, 128 * (p + 1))
                    lhsT = YF[:, 256 * p : 256 * p + 128]
                    flush0 = 0
                    pend = 0
                    for gi, g in enumerate(groups):
                        ps = mpsum.tile([128, GW * CH], F32, tag="ps")
                        for qi, j in enumerate(g):
                            nc.tensor.matmul(
                                ps[:, qi * CH : (qi + 1) * CH],
                                lhsT=lhsT,
                                rhs=YF[:, j * CH : (j + 1) * CH],
                                start=True,
                                stop=True,
                            )
                        lc = slice((g[0] - c) * CH, (g[0] - c + len(g)) * CH)
                        quant_copy(pnl[:, lc], ps[:, : len(g) * CH], len(g) * CH)
                        pend += len(g) * CH
                        if pend >= 3 * CH or gi == len(groups) - 1:
                            fl = slice(flush0, flush0 + pend)
                            nc.sync.dma_start(out=out[rs_, fl], in_=pnl[:, fl])
                            flush0 += pend
                            pend = 0

            # Software-pipelined: panels for chunk c are emitted after the
            # norm of chunk c-1 so the norm chain never queues behind the
            # bulk matmul/copy work on DVE/ACT/PE.
            norm_chunk(NCH - 1)
            for c in range(NCH - 2, -1, -1):
                norm_chunk(c)
                emit_panels(c + 1)
            emit_panels(0)

    nc.compile()
    return nc


def _install_profile_hook():
    """This container's antenv lacks axon_hooks, so run_bass_kernel_spmd's
    trace=True path dies on import. Recreate the module and register the
    ctypes NTFF hook that trn_boot would have installed."""
    import sys as _sys
    import types

    if "antenv.axon_hooks" in _sys.modules:
        return
    import antenv

    mod = types.ModuleType("antenv.axon_hooks")
    mod._hook = None

    def set_axon_ntff_profile_hook(h):
        mod._hook = h

    def get_axon_ntff_profile_hook():
        return mod._hook

    mod.set_axon_ntff_profile_hook = set_axon_ntff_profile_hook
    mod.get_axon_ntff_profile_hook = get_axon_ntff_profile_hook
    _sys.modules["antenv.axon_hooks"] = mod
    antenv.axon_hooks = mod

    from trn_agent_boot.trn_boot import _ntff_profile_via_ctypes

    mod.set_axon_ntff_profile_hook(
        _ntff_profile_via_ctypes("/opt/axon/libaxon_pjrt.so")
    )


_nc = None


def _get_nc():
    global _nc
    if _nc is None:
        _nc = _build()
    return _nc


def _run(x, trace=False, trace_cores=None):
    x = np.asarray(x, dtype=np.float32)
    assert x.shape == (B, C, N), x.shape
    core_ids = list(range(NCORES))
    in_maps = []
    for k in core_ids:
        b, r = divmod(k, 2)
        xb = x[b] if r == 0 else np.roll(x[b], -128, axis=1)
        xb16 = xb.astype(np.float16)
        in_maps.append({"xf": np.ascontiguousarray(np.vstack([xb16, xb16]))})
    if trace:
        _install_profile_hook()
    res = run_bass_kernel_spmd(
        _get_nc(), in_maps, core_ids, trace=trace, trace_cores=trace_cores
    )
    out = np.empty((B, N, N), dtype=np.float32)
    for k in core_ids:
        b, r = divmod(k, 2)
        S = res.results[k]["out"]  # [2048, 4096] fp16
        Sf = S.astype(np.float32)
        for p in range(16):
            t = 2 * p + r
            ss = 512 * (p // 2)  # chunk-aligned col start (shifted coords)
            L = (N - ss) - 128 * r  # valid slab length (clip wraparound)
            cs = ss + 128 * r  # actual col start
            out[b, 128 * t : 128 * (t + 1), cs : cs + L] = Sf[
                128 * p : 128 * (p + 1), 0:L
            ]
    # Mirror the block lower triangle from the computed upper part.
    for b in range(B):
        ob = out[b]
        for t in range(1, 32):
            fs = 512 * (t // 4) + 128 * (t % 2)
            if fs:
                ob[128 * t : 128 * (t + 1), 0:fs] = ob[
                    0:fs, 128 * t : 128 * (t + 1)
                ].T
    return out, res


def kernel(x):
    return _run(x)[0]


# revision 17
# speedup vs baseline: 1.0874x; 1.0182x over previous
"""Cosine-similarity attention map on 8 Trainium2 NeuronCores.

out[b, i, j] = <x[b,:,i], x[b,:,j]> / (||x[b,:,i]|| * ||x[b,:,j]||)
x: [B=4, C=64, N=4096] fp32  ->  out: [B=4, N=4096, N=4096] fp32

The output is a symmetric Gram matrix, so each core computes only its
share of the (block) upper triangle, in fp16, and the host mirrors the
lower triangle while unsharding (rel tolerance 2e-2; fp16 costs ~5e-4).

Sharding: 2 cores per batch. Global 128-row panels t = 0..31 of out[b];
core r in {0,1} owns panels t = 2p + r (p = 0..15 local). Panel t only
needs columns >= 128t; rounding down to 512-col chunks, local panel p
computes chunks floor(p/2)..7, width w = 8 - floor(p/2) — identical for
both cores, so one SPMD program serves all 8. Row data comes from the
same normalized tensor as column data (rows == cols of a Gram matrix):
core r receives x[b] rolled left by 128*r columns. The channel rows are
host-duplicated to K=128 ([x; x] doubles sumsq, and rsqrt then yields
exactly the extra 1/sqrt(2) each copy needs): a full-height contraction
keeps the PE's HAM activity monitor in its warm state (2.4 GHz); a
12-matmul dependency-free burst warms it up front.
"""

import sys

sys.path.insert(0, "/opt/trn_rl_repo")

import numpy as np

import concourse.bass as bass
import concourse.mybir as mybir
import concourse.tile as tile
from concourse import bacc
from concourse.bass_utils import run_bass_kernel_spmd

B, C, N = 4, 64, 4096
NCORES = 8
RB = 2048  # 16 local 128-row panels per core
CH = 512  # norm / matmul column chunk
NCH = N // CH  # 8
GW = 2  # chunks per PSUM group (copy width 1024)

F32 = mybir.dt.float32
F16 = mybir.dt.float16
I8 = mybir.dt.int8


def _build():
    nc = bacc.Bacc("TRN2", target_bir_lowering=False)
    xf = nc.declare_dram_parameter("xf", [2 * C, N], F16, isOutput=False)
    out16 = nc.declare_dram_parameter("out16", [RB, N], F16, isOutput=True)
    out8 = nc.declare_dram_parameter("out8", [RB, N], I8, isOutput=True)


    with tile.TileContext(nc) as tc:
        with (
            tc.tile_pool(name="persist", bufs=1) as persist,
            tc.tile_pool(name="panels", bufs=3) as panels,
            tc.tile_pool(name="panels8", bufs=3) as panels8,
            tc.tile_pool(name="mpsum", bufs=3, space="PSUM") as mpsum,
            tc.tile_pool(name="npsum", bufs=2, space="PSUM") as npsum,
        ):
            # PE warm-up: 12 dependency-free matmuls (~7us cold, two full HAM
            # windows) flip the clock gate to 2.4 GHz before the real matmuls.
            # They use the first mpsum ring slot before any panel claims it.
            GARB = persist.tile([2 * C, CH], F16)
            nc.vector.memset(GARB, 0.5)
            WPS = mpsum.tile([128, GW * CH], F32, tag="ps")
            for _ in range(12):
                nc.tensor.matmul(
                    WPS[:, 0:CH], lhsT=GARB[:, 0:128], rhs=GARB, start=True, stop=True
                )

            XF = persist.tile([2 * C, N], F16)
            # Chunks are consumed descending (small panels first), so load
            # them in that order too, two chunks per DMA.
            for c in range(NCH - 2, -1, -2):
                cs = slice(c * CH, (c + 2) * CH)
                nc.sync.dma_start(out=XF[:, cs], in_=xf[:, cs])

            ones_f = persist.tile([2 * C, 1], F32)
            nc.vector.memset(ones_f, 1.0)
            ones_c = persist.tile([2 * C, 1], F16)  # sumsq reduction lhsT
            nc.vector.tensor_copy(ones_c, ones_f)
            ones_rf = persist.tile([1, 2 * C], F32)
            nc.vector.memset(ones_rf, 1.0)
            ones_r = persist.tile([1, 2 * C], F16)  # K=1 partition-broadcast lhsT
            nc.vector.tensor_copy(ones_r, ones_rf)

            SQ = persist.tile([2 * C, N], F16)
            RN16 = persist.tile([1, N], F16)
            YF = persist.tile([2 * C, N], F16)

            # x^2 on the otherwise-idle GpSimd, descending, as loads land.
            for c in range(NCH - 1, -1, -1):
                cs = slice(c * CH, (c + 1) * CH)
                nc.gpsimd.tensor_mul(SQ[:, cs], XF[:, cs], XF[:, cs])

            # Normalize columns of one 512-col chunk: y = x * rsqrt(sumsq).
            def norm_chunk(c):
                cs = slice(c * CH, (c + 1) * CH)
                pps = npsum.tile([128, CH], F32, tag="pps")
                nc.tensor.matmul(
                    pps[0:1, :], lhsT=ones_c, rhs=SQ[:, cs], start=True, stop=True
                )
                nc.scalar.activation(
                    RN16[:, cs],
                    pps[0:1, :],
                    mybir.ActivationFunctionType.Abs_reciprocal_sqrt,
                )
                nc.tensor.matmul(
                    pps[0 : 2 * C, :], lhsT=ones_r, rhs=RN16[:, cs], start=True, stop=True
                )
                nc.vector.tensor_mul(YF[:, cs], XF[:, cs], pps[0 : 2 * C, :])

            # Panels 2c and 2c+1: rhs chunks c..7, lhsT inside chunk c.
            # Even panels ship fp16 (DVE evacuation, fast 16-bit store);
            # odd panels ship int8 = round(127*cos) (ACT evacuation with
            # the scale fused into the copy) — each stage loads both copy
            # engines evenly and output traffic drops by a quarter.
            def emit_panels(c):
                js = list(range(c, NCH))
                groups = [js[i : i + GW] for i in range(0, len(js), GW)]
                for p in (2 * c, 2 * c + 1):
                    is8 = p % 2 == 1
                    if is8:
                        pnl = panels8.tile([128, N], I8, tag="panel8")
                        dst = out8
                    else:
                        pnl = panels.tile([128, N], F16, tag="panel")
                        dst = out16
                    rs_ = slice(128 * p, 128 * (p + 1))
                    lhsT = YF[:, 256 * p : 256 * p + 128]
                    flush0 = 0
                    pend = 0
                    for gi, g in enumerate(groups):
                        ps = mpsum.tile([128, GW * CH], F32, tag="ps")
                        for qi, j in enumerate(g):
                            nc.tensor.matmul(
                                ps[:, qi * CH : (qi + 1) * CH],
                                lhsT=lhsT,
                                rhs=YF[:, j * CH : (j + 1) * CH],
                                start=True,
                                stop=True,
                            )
                        lc = slice((g[0] - c) * CH, (g[0] - c + len(g)) * CH)
                        if is8:
                            nc.scalar.activation(
                                pnl[:, lc], ps[:, : len(g) * CH],
                                mybir.ActivationFunctionType.Copy,
                                bias=0.0, scale=127.0,
                            )
                        else:
                            nc.vector.tensor_copy(pnl[:, lc], ps[:, : len(g) * CH])
                        pend += len(g) * CH
                        if pend >= 3 * CH or gi == len(groups) - 1:
                            fl = slice(flush0, flush0 + pend)
                            nc.sync.dma_start(out=dst[rs_, fl], in_=pnl[:, fl])
                            flush0 += pend
                            pend = 0

            # Software-pipelined: panels for chunk c are emitted after the
            # norm of chunk c-1 so the norm chain never queues behind the
            # bulk matmul/copy work on DVE/ACT/PE.
            norm_chunk(NCH - 1)
            for c in range(NCH - 2, -1, -1):
                norm_chunk(c)
                emit_panels(c + 1)
            emit_panels(0)

    nc.compile()
    return nc


def _install_profile_hook():
    """This container's antenv lacks axon_hooks, so run_bass_kernel_spmd's
    trace=True path dies on import. Recreate the module and register the
    ctypes NTFF hook that trn_boot would have installed."""
    import sys as _sys
    import types

    if "antenv.axon_hooks" in _sys.modules:
        return
    import antenv

    mod = types.ModuleType("antenv.axon_hooks")
    mod._hook = None

    def set_axon_ntff_profile_hook(h):
        mod._hook = h

    def get_axon_ntff_profile_hook():
        return mod._hook

    mod.set_axon_ntff_profile_hook = set_axon_ntff_profile_hook
    mod.get_axon_ntff_profile_hook = get_axon_ntff_profile_hook
    _sys.modules["antenv.axon_hooks"] = mod
    antenv.axon_hooks = mod

    from trn_agent_boot.trn_boot import _ntff_profile_via_ctypes

    mod.set_axon_ntff_profile_hook(
        _ntff_profile_via_ctypes("/opt/axon/libaxon_pjrt.so")
    )


_nc = None


def _get_nc():
    global _nc
    if _nc is None:
        _nc = _build()
    return _nc


def _run(x, trace=False, trace_cores=None):
    x = np.asarray(x, dtype=np.float32)
    assert x.shape == (B, C, N), x.shape
    core_ids = list(range(NCORES))
    in_maps = []
    for k in core_ids:
        b, r = divmod(k, 2)
        xb = x[b] if r == 0 else np.roll(x[b], -128, axis=1)
        xb16 = xb.astype(np.float16)
        in_maps.append({"xf": np.ascontiguousarray(np.vstack([xb16, xb16]))})
    if trace:
        _install_profile_hook()
    res = run_bass_kernel_spmd(
        _get_nc(), in_maps, core_ids, trace=trace, trace_cores=trace_cores
    )
    out = np.empty((B, N, N), dtype=np.float32)
    for k in core_ids:
        b, r = divmod(k, 2)
        S16 = res.results[k]["out16"]  # even panels, fp16
        S8 = res.results[k]["out8"]  # odd panels, int8 = 127*cos
        for p in range(16):
            if p % 2:
                Sf = S8.astype(np.float32) * (1.0 / 127.0)
            else:
                Sf = S16.astype(np.float32)
            t = 2 * p + r
            ss = 512 * (p // 2)  # chunk-aligned col start (shifted coords)
            L = (N - ss) - 128 * r  # valid slab length (clip wraparound)
            cs = ss + 128 * r  # actual col start
            out[b, 128 * t : 128 * (t + 1), cs : cs + L] = Sf[
                128 * p : 128 * (p + 1), 0:L
            ]
    # Mirror the block lower triangle from the computed upper part.
    for b in range(B):
        ob = out[b]
        for t in range(1, 32):
            fs = 512 * (t // 4) + 128 * (t % 2)
            if fs:
                ob[128 * t : 128 * (t + 1), 0:fs] = ob[
                    0:fs, 128 * t : 128 * (t + 1)
                ].T
    return out, res


def kernel(x):
    return _run(x)[0]


# revision 18
# speedup vs baseline: 1.1170x; 1.0272x over previous
"""Cosine-similarity attention map on 8 Trainium2 NeuronCores.

out[b, i, j] = <x[b,:,i], x[b,:,j]> / (||x[b,:,i]|| * ||x[b,:,j]||)
x: [B=4, C=64, N=4096] fp32  ->  out: [B=4, N=4096, N=4096] fp32

The output is a symmetric Gram matrix, so each core computes only its
share of the (block) upper triangle, in fp16, and the host mirrors the
lower triangle while unsharding (rel tolerance 2e-2; fp16 costs ~5e-4).

Sharding: 2 cores per batch. Global 128-row panels t = 0..31 of out[b];
core r in {0,1} owns panels t = 2p + r (p = 0..15 local). Panel t only
needs columns >= 128t; rounding down to 512-col chunks, local panel p
computes chunks floor(p/2)..7, width w = 8 - floor(p/2) — identical for
both cores, so one SPMD program serves all 8. Row data comes from the
same normalized tensor as column data (rows == cols of a Gram matrix):
core r receives x[b] rolled left by 128*r columns. The channel rows are
host-duplicated to K=128 ([x; x] doubles sumsq, and rsqrt then yields
exactly the extra 1/sqrt(2) each copy needs): a full-height contraction
keeps the PE's HAM activity monitor in its warm state (2.4 GHz); a
12-matmul dependency-free burst warms it up front.
"""

import sys

sys.path.insert(0, "/opt/trn_rl_repo")

import numpy as np

import concourse.bass as bass
import concourse.mybir as mybir
import concourse.tile as tile
from concourse import bacc
from concourse.bass_utils import run_bass_kernel_spmd

B, C, N = 4, 64, 4096
NCORES = 8
RB = 2048  # 16 local 128-row panels per core
CH = 512  # norm / matmul column chunk
NCH = N // CH  # 8
GW = 2  # chunks per PSUM group (copy width 1024)

F32 = mybir.dt.float32
F16 = mybir.dt.float16


def _build():
    nc = bacc.Bacc("TRN2", target_bir_lowering=False)
    xf = nc.declare_dram_parameter("xf", [2 * C, N], F16, isOutput=False)
    out = nc.declare_dram_parameter("out", [RB, N], F16, isOutput=True)

    # Projected busy time (us) per copy engine; DVE (0.96 GHz) also does
    # the 8 chunk muls, ACT (1.2 GHz) the 8 abs_rsqrt.
    eng_t = {"v": 5.5, "a": 6.0}

    with tile.TileContext(nc) as tc:
        with (
            tc.tile_pool(name="persist", bufs=1) as persist,
            tc.tile_pool(name="panels", bufs=6) as panels,
            tc.tile_pool(name="mpsum", bufs=3, space="PSUM") as mpsum,
            tc.tile_pool(name="npsum", bufs=2, space="PSUM") as npsum,
        ):
            # PE warm-up: 12 dependency-free matmuls (~7us cold, two full HAM
            # windows) flip the clock gate to 2.4 GHz before the real matmuls.
            # They use the first mpsum ring slot before any panel claims it.
            GARB = persist.tile([2 * C, CH], F16)
            nc.vector.memset(GARB, 0.5)
            WPS = mpsum.tile([128, GW * CH], F32, tag="ps")
            for _ in range(12):
                nc.tensor.matmul(
                    WPS[:, 0:CH], lhsT=GARB[:, 0:128], rhs=GARB, start=True, stop=True
                )

            XF = persist.tile([2 * C, N], F16)
            # Chunks are consumed descending (small panels first), so load
            # them in that order too, two chunks per DMA.
            for c in range(NCH - 2, -1, -2):
                cs = slice(c * CH, (c + 2) * CH)
                nc.sync.dma_start(out=XF[:, cs], in_=xf[:, cs])

            ones_f = persist.tile([2 * C, 1], F32)
            nc.vector.memset(ones_f, 1.0)
            ones_c = persist.tile([2 * C, 1], F16)  # sumsq reduction lhsT
            nc.vector.tensor_copy(ones_c, ones_f)
            ones_rf = persist.tile([1, 2 * C], F32)
            nc.vector.memset(ones_rf, 1.0)
            ones_r = persist.tile([1, 2 * C], F16)  # K=1 partition-broadcast lhsT
            nc.vector.tensor_copy(ones_r, ones_rf)

            SQ = persist.tile([2 * C, N], F16)
            RN16 = persist.tile([1, N], F16)
            YF = persist.tile([2 * C, N], F16)

            # x^2 on the otherwise-idle GpSimd, descending, as loads land.
            for c in range(NCH - 1, -1, -1):
                cs = slice(c * CH, (c + 1) * CH)
                nc.gpsimd.tensor_mul(SQ[:, cs], XF[:, cs], XF[:, cs])

            # Normalize columns of one 512-col chunk: y = x * rsqrt(sumsq).
            def norm_chunk(c):
                cs = slice(c * CH, (c + 1) * CH)
                pps = npsum.tile([128, CH], F32, tag="pps")
                nc.tensor.matmul(
                    pps[0:1, :], lhsT=ones_c, rhs=SQ[:, cs], start=True, stop=True
                )
                nc.scalar.activation(
                    RN16[:, cs],
                    pps[0:1, :],
                    mybir.ActivationFunctionType.Abs_reciprocal_sqrt,
                )
                nc.tensor.matmul(
                    pps[0 : 2 * C, :], lhsT=ones_r, rhs=RN16[:, cs], start=True, stop=True
                )
                nc.vector.tensor_mul(YF[:, cs], XF[:, cs], pps[0 : 2 * C, :])

            # PSUM -> SBUF evacuation, fp16 out (int8 measured SLOWER here:
            # the 8-bit store path runs below the 1 elem/cycle cast rate).
            def quant_copy(dst, src, cols):
                if eng_t["v"] <= eng_t["a"]:
                    eng_t["v"] += 0.105 + cols / 1010.0
                    nc.vector.tensor_copy(dst, src)
                else:
                    eng_t["a"] += 0.125 + cols / 1030.0
                    nc.scalar.copy(out=dst, in_=src)

            # Panels 2c and 2c+1: rhs chunks c..7, lhsT inside chunk c.
            def emit_panels(c):
                js = list(range(c, NCH))
                groups = [js[i : i + GW] for i in range(0, len(js), GW)]
                for p in (2 * c, 2 * c + 1):
                    pnl = panels.tile([128, N], F16, tag="panel")
                    rs_ = slice(128 * p, 128 * (p + 1))
                    lhsT = YF[:, 256 * p : 256 * p + 128]
                    flush0 = 0
                    pend = 0
                    for gi, g in enumerate(groups):
                        ps = mpsum.tile([128, GW * CH], F32, tag="ps")
                        for qi, j in enumerate(g):
                            nc.tensor.matmul(
                                ps[:, qi * CH : (qi + 1) * CH],
                                lhsT=lhsT,
                                rhs=YF[:, j * CH : (j + 1) * CH],
                                start=True,
                                stop=True,
                            )
                        lc = slice((g[0] - c) * CH, (g[0] - c + len(g)) * CH)
                        quant_copy(pnl[:, lc], ps[:, : len(g) * CH], len(g) * CH)
                        pend += len(g) * CH
                        if pend >= 3 * CH or gi == len(groups) - 1:
                            fl = slice(flush0, flush0 + pend)
                            nc.sync.dma_start(out=out[rs_, fl], in_=pnl[:, fl])
                            flush0 += pend
                            pend = 0

            # Software-pipelined: panels for chunk c are emitted after the
            # norm of chunk c-1 so the norm chain never queues behind the
            # bulk matmul/copy work on DVE/ACT/PE.
            norm_chunk(NCH - 1)
            for c in range(NCH - 2, -1, -1):
                norm_chunk(c)
                emit_panels(c + 1)
            emit_panels(0)

    nc.compile()
    return nc


def _install_profile_hook():
    """This container's antenv lacks axon_hooks, so run_bass_kernel_spmd's
    trace=True path dies on import. Recreate the module and register the
    ctypes NTFF hook that trn_boot would have installed."""
    import sys as _sys
    import types

    if "antenv.axon_hooks" in _sys.modules:
        return
    import antenv

    mod = types.ModuleType("antenv.axon_hooks")
    mod._hook = None

    def set_axon_ntff_profile_hook(h):
        mod._hook = h

    def get_axon_ntff_profile_hook():
        return mod._hook

    mod.set_axon_ntff_profile_hook = set_axon_ntff_profile_hook
    mod.get_axon_ntff_profile_hook = get_axon_ntff_profile_hook
    _sys.modules["antenv.axon_hooks"] = mod
    antenv.axon_hooks = mod

    from trn_agent_boot.trn_boot import _ntff_profile_via_ctypes

    mod.set_axon_ntff_profile_hook(
        _ntff_profile_via_ctypes("/opt/axon/libaxon_pjrt.so")
    )


_nc = None


def _get_nc():
    global _nc
    if _nc is None:
        _nc = _build()
    return _nc


def _run(x, trace=False, trace_cores=None):
    x = np.asarray(x, dtype=np.float32)
    assert x.shape == (B, C, N), x.shape
    core_ids = list(range(NCORES))
    in_maps = []
    for k in core_ids:
        b, r = divmod(k, 2)
        xb = x[b] if r == 0 else np.roll(x[b], -128, axis=1)
        xb16 = xb.astype(np.float16)
        in_maps.append({"xf": np.ascontiguousarray(np.vstack([xb16, xb16]))})
    if trace:
        _install_profile_hook()
    res = run_bass_kernel_spmd(
        _get_nc(), in_maps, core_ids, trace=trace, trace_cores=trace_cores
    )
    out = np.empty((B, N, N), dtype=np.float32)
    for k in core_ids:
        b, r = divmod(k, 2)
        S = res.results[k]["out"]  # [2048, 4096] fp16
        Sf = S.astype(np.float32)
        for p in range(16):
            t = 2 * p + r
            ss = 512 * (p // 2)  # chunk-aligned col start (shifted coords)
            L = (N - ss) - 128 * r  # valid slab length (clip wraparound)
            cs = ss + 128 * r  # actual col start
            out[b, 128 * t : 128 * (t + 1), cs : cs + L] = Sf[
                128 * p : 128 * (p + 1), 0:L
            ]
    # Mirror the block lower triangle from the computed upper part.
    for b in range(B):
        ob = out[b]
        for t in range(1, 32):
            fs = 512 * (t // 4) + 128 * (t % 2)
            if fs:
                ob[128 * t : 128 * (t + 1), 0:fs] = ob[
                    0:fs, 128 * t : 128 * (t + 1)
                ].T
    return out, res


def kernel(x):
    return _run(x)[0]
